# revision 1
# baseline (speedup 1.0000x reference)
"""NonLocalAttention2D Trainium2 kernel.

Data-parallel over batch N=8: one image per NeuronCore.

Per-core math (x: (C=128, HW=4096) fp32):
  kv   = [Wv|Wk].T @ x                     (80, 4096)   PE
  pool = maxpool2x2(kv)                    (80, 1024)   DVE (k rows 64:80, v rows 0:64)
  A_c  = Wq @ k_c                          (128, 128)   PE   (8 key chunks of 128)
  scores_c(b) = A_c.T @ x_b                (128k, 512q) PE   -> psum
  attn = exp(scores)                       ACT psum->sbuf (no max-sub; |s|<~60 safe in fp32)
  av   = [v*e^{k.bq} | e^{k.bq}].T @ attn  (65, 512)    PE   row 64 = softmax denominators
  aoTn = av * broadcast(1/denoms)          DVE (recip + DMA partition-broadcast)
  fin  = [g*Wo; g*bo].T @ aoTn             (128, 512)   PE
  out  = fin + x_b                         DVE -> DMA out
"""

import sys

if "/opt/trn_rl_repo" not in sys.path:
    sys.path.insert(0, "/opt/trn_rl_repo")

import numpy as np

import concourse.bacc as bacc
import concourse.bass as bass
import concourse.tile as tile
from concourse import bass_utils, masks, mybir

F32 = mybir.dt.float32
BF16 = mybir.dt.bfloat16
F32R = mybir.dt.float32r


def _r(ap):
    return ap.bitcast(F32R)

C = 128          # channels
HW = 4096        # 64*64 pixels
L = 1024         # pooled keys (32*32)
D = 16           # attn dim
DV = 64          # value dim
QB = 512         # q-block size
NB = HW // QB    # 8 q blocks
KC = 128         # keys per chunk
NC_CHUNKS = L // KC  # 8 key chunks
NCORES = 8


def build_kernel(variant="full"):
    nc = bacc.Bacc("TRN2", target_bir_lowering=False, debug=False)

    x_d = nc.dram_tensor("x", (C, HW), F32, kind="ExternalInput").ap()
    wkv_d = nc.dram_tensor("wkv", (C, 80), F32, kind="ExternalInput").ap()
    wqt_d = nc.dram_tensor("wqt", (D, C), F32, kind="ExternalInput").ap()
    wfin_d = nc.dram_tensor("wfin", (DV + 1, C), F32, kind="ExternalInput").ap()
    bkv_d = nc.dram_tensor("bkv", (80, 1), F32, kind="ExternalInput").ap()
    bq_d = nc.dram_tensor("bq", (D, 1), F32, kind="ExternalInput").ap()
    out_d = nc.dram_tensor("out", (C, HW), F32, kind="ExternalOutput").ap()

    from contextlib import ExitStack

    with tile.TileContext(nc) as tc, ExitStack() as ctx:
        singles = ctx.enter_context(tc.tile_pool(name="singles", bufs=1))
        s1_pool = ctx.enter_context(tc.tile_pool(name="s1", bufs=2))
        attn_pool = ctx.enter_context(tc.tile_pool(name="attn", bufs=2))
        r_pool = ctx.enter_context(tc.tile_pool(name="r", bufs=2))
        R_pool = ctx.enter_context(tc.tile_pool(name="R", bufs=2))
        ao_pool = ctx.enter_context(tc.tile_pool(name="ao", bufs=2))
        out_pool = ctx.enter_context(tc.tile_pool(name="outp", bufs=3))
        dram_pool = ctx.enter_context(tc.tile_pool(name="dram", bufs=2, space="DRAM"))

        ps_score = ctx.enter_context(tc.tile_pool(name="ps_score", bufs=2, space="PSUM"))
        ps_av = ctx.enter_context(tc.tile_pool(name="ps_av", bufs=2, space="PSUM"))
        ps_fin = ctx.enter_context(tc.tile_pool(name="ps_fin", bufs=2, space="PSUM"))

        # ---- constants / weights in SBUF ----
        w_kv = singles.tile([C, 80], F32R, tag="wkv")
        w_qt = singles.tile([D, C], F32R, tag="wqt")
        w_fin = singles.tile([DV + 1, C], F32R, tag="wfin")
        b_kv = singles.tile([80, 1], F32, tag="bkv")
        b_q = singles.tile([D, 1], F32R, tag="bq")
        ident = singles.tile([DV, DV], F32, tag="ident")
        nc.sync.dma_start(out=w_kv, in_=wkv_d.bitcast(F32R))
        nc.sync.dma_start(out=w_qt, in_=wqt_d.bitcast(F32R))
        nc.sync.dma_start(out=w_fin, in_=wfin_d.bitcast(F32R))
        nc.sync.dma_start(out=b_kv, in_=bkv_d)
        nc.sync.dma_start(out=b_q, in_=bq_d.bitcast(F32R))
        masks.make_identity(nc, ident[:, :])

        x_sb = singles.tile([C, HW], F32R, tag="x")
        kv_pool = singles.tile([80, L], F32, tag="kvp")
        k_sb = singles.tile([D, L], F32R, tag="k")
        a_sb = singles.tile([C, NC_CHUNKS * KC], F32R, tag="a")
        vaug_sb = singles.tile([KC, NC_CHUNKS * (DV + 1)], BF16, tag="vaug")
        ebqk_sb = singles.tile([KC, NC_CHUNKS], F32, tag="ebqk")

        # ---- prologue: load x, project k/v, pool ----
        for c in range(NB):
            sl = slice(c * QB, (c + 1) * QB)
            nc.sync.dma_start(out=x_sb[:, sl], in_=x_d[:, sl].bitcast(F32R))
            proj = ps_fin.tile([C, QB], F32, tag="fin")
            nc.tensor.matmul(
                proj[:80, :], lhsT=w_kv[:, :], rhs=x_sb[:, sl], start=True, stop=True
            )
            # maxpool step 1: adjacent w pairs. view (80, 512) as (80, 256, 2)
            pv = proj[:80, :].rearrange("p (w two) -> p w two", two=2)
            s1 = s1_pool.tile([80, 256], F32, tag="s1")
            nc.vector.tensor_copy(s1[:, :], pv[:, :, 0])
            nc.vector.tensor_max(s1[:, :], s1[:, :], pv[:, :, 1])
            # maxpool step 2: h pairs. s1 is (80, 4h2, 32w) flat; pairs 32 apart
            sv = s1.rearrange("p (h two w) -> p h two w", h=4, two=2)
            ov = kv_pool[:, c * KC : (c + 1) * KC].rearrange("p (h w) -> p h w", h=4)
            nc.vector.tensor_max(ov, sv[:, :, 0, :], sv[:, :, 1, :])

        # bias add on pooled k/v (bv rows 0:64, bk rows 64:80)
        nc.vector.tensor_scalar_add(kv_pool[:, :], kv_pool[:, :], b_kv[:, :])
        # move k rows to partition base 0
        if variant == "nokdma":
            nc.vector.memset(k_sb[:, :], 1.0)
        else:
            nc.sync.dma_start(out=k_sb[:, :], in_=kv_pool[64:80, :].bitcast(F32R))

        # A_c = Wq @ k_c ; bqk_c = k_c.T @ bq
        bqk = ps_fin.tile([KC, NC_CHUNKS], F32, tag="fin")
        for c in range(NC_CHUNKS):
            if variant == "nobqk":
                ksl = slice(c * KC, (c + 1) * KC)
                a_ps = ps_av.tile([C, KC], F32, tag="av")
                nc.tensor.matmul(
                    a_ps[:, :], lhsT=w_qt[:, :], rhs=k_sb[:, ksl], start=True, stop=True
                )
                nc.vector.tensor_copy(a_sb[:, ksl], a_ps[:, :])
                continue
            ksl = slice(c * KC, (c + 1) * KC)
            a_ps = ps_av.tile([C, KC], F32, tag="av")
            nc.tensor.matmul(
                a_ps[:, :], lhsT=w_qt[:, :], rhs=k_sb[:, ksl], start=True, stop=True
            )
            nc.vector.tensor_copy(a_sb[:, ksl], a_ps[:, :])
            nc.tensor.matmul(
                bqk[:, c : c + 1],
                lhsT=k_sb[:, ksl].bitcast(F32),
                rhs=b_q[:, :].bitcast(F32),
                start=(c == 0),
                stop=(c == NC_CHUNKS - 1),
                skip_group_check=True,
            )
        if variant == "nobqk":
            nc.vector.memset(ebqk_sb[:, :], 1.0)
        else:
            nc.scalar.activation(
                ebqk_sb[:, :], bqk[:, :], mybir.ActivationFunctionType.Exp
            )

        # vT chunks via PE transpose, scaled by e^{bqk}; col 0 of each group = e^{bqk}
        for c in range(NC_CHUNKS):
            vt_ps = ps_av.tile([KC, DV], F32, tag="av")
            nc.tensor.transpose(
                vt_ps[:, :], kv_pool[:DV, c * KC : (c + 1) * KC], ident[:, :]
            )
            base = c * (DV + 1)
            nc.vector.tensor_scalar_mul(
                vaug_sb[:, base : base + DV], vt_ps[:, :], ebqk_sb[:, c : c + 1]
            )
            nc.vector.tensor_copy(
                vaug_sb[:, base + DV : base + DV + 1], ebqk_sb[:, c : c + 1]
            )

        if variant == "prologue":
            nc.sync.dma_start(out=out_d[:, 0:1024], in_=a_sb[:, :].bitcast(F32))
            nc.sync.dma_start(
                out=out_d[:80, 1024:2048], in_=kv_pool[:, :]
            )
            nc.sync.dma_start(
                out=out_d[:, 2048:2080], in_=vaug_sb[:, 0:32].bitcast(mybir.dt.uint16).bitcast(F32)
            )
        # ---- main loop over q blocks ----
        for b in range(NB if variant != "prologue" else 0):
            qsl = slice(b * QB, (b + 1) * QB)
            attn = attn_pool.tile([KC, NC_CHUNKS * QB], BF16, tag="attn")
            for t in range(4):  # 4 score tiles of 2 chunks each
                sc = ps_score.tile([KC, 2 * QB], F32, tag="sc")
                for j in range(2):
                    c = 2 * t + j
                    nc.tensor.matmul(
                        sc[:, j * QB : (j + 1) * QB],
                        lhsT=a_sb[:, c * KC : (c + 1) * KC],
                        rhs=x_sb[:, qsl],
                        start=True,
                        stop=True,
                    )
                nc.scalar.activation(
                    attn[:, t * 2 * QB : (t + 1) * 2 * QB],
                    sc[:, :],
                    mybir.ActivationFunctionType.Exp,
                )
            av = ps_av.tile([DV + 1, QB], F32, tag="av")
            for c in range(NC_CHUNKS):
                base = c * (DV + 1)
                nc.tensor.matmul(
                    av[:, :],
                    lhsT=vaug_sb[:, base : base + DV + 1],
                    rhs=attn[:, c * QB : (c + 1) * QB],
                    start=(c == 0),
                    stop=(c == NC_CHUNKS - 1),
                )
            R65 = R_pool.tile([DV + 1, QB], F32, tag="R")
            if variant == "noR":
                nc.vector.memset(R65[:, :], 1.0)
            else:
                r = r_pool.tile([1, QB], F32, tag="r")
                nc.vector.reciprocal(r[:, :], av[DV : DV + 1, :])
                # broadcast r across 65 partitions (bounce via DRAM: DMA reads
                # the row 65 times with partition stride 0)
                r_dram = dram_pool.tile([1, QB], F32, tag="rd")
                nc.sync.dma_start(out=r_dram[:, :], in_=r[:, :])
                r_bcast = bass.AP(
                    tensor=r_dram.tensor, offset=r_dram.offset, ap=[[0, DV + 1], [1, QB]]
                )
                nc.sync.dma_start(out=R65[:, :], in_=r_bcast)
            aoTn = ao_pool.tile([DV + 1, QB], F32R, tag="ao")
            nc.vector.tensor_mul(aoTn[:, :], av[:, :], R65[:, :])
            fin = ps_fin.tile([C, QB], F32, tag="fin")
            nc.tensor.matmul(
                fin[:, :], lhsT=w_fin[:, :], rhs=aoTn[:, :], start=True, stop=True
            )
            o_sb = out_pool.tile([C, QB], F32, tag="o")
            nc.vector.tensor_add(o_sb[:, :], fin[:, :], x_sb[:, qsl].bitcast(F32))
            nc.sync.dma_start(out=out_d[:, qsl], in_=o_sb[:, :])

    nc.compile()
    return nc


def prep_weights(Wq, bq, Wk, bk, Wv, bv, Wo, bo, gamma):
    g = np.float32(np.asarray(gamma))
    wkv = np.concatenate([np.asarray(Wv), np.asarray(Wk)], axis=1).astype(np.float32)
    wkv = np.ascontiguousarray(wkv)  # (128, 80): v cols 0:64, k cols 64:80
    wqt = np.ascontiguousarray(np.asarray(Wq).T.astype(np.float32))  # (16, 128)
    wfin = np.concatenate(
        [g * np.asarray(Wo), (g * np.asarray(bo))[None, :]], axis=0
    ).astype(np.float32)  # (65, 128)
    bkv = np.concatenate([np.asarray(bv), np.asarray(bk)])[:, None].astype(np.float32)
    bq_ = np.asarray(bq)[:, None].astype(np.float32)
    return wkv, wqt, wfin, bkv, bq_


_NC_CACHE = {}


def kernel(x, Wq, bq, Wk, bk, Wv, bv, Wo, bo, gamma):
    x = np.asarray(x, dtype=np.float32)
    N = x.shape[0]
    assert x.shape == (N, C, 64, 64) and N == NCORES
    wkv, wqt, wfin, bkv, bq_ = prep_weights(Wq, bq, Wk, bk, Wv, bv, Wo, bo, gamma)

    if "nc" not in _NC_CACHE:
        _NC_CACHE["nc"] = build_kernel()
    nc = _NC_CACHE["nc"]

    in_maps = []
    for i in range(N):
        in_maps.append(
            {
                "x": np.ascontiguousarray(x[i].reshape(C, HW)),
                "wkv": wkv,
                "wqt": wqt,
                "wfin": wfin,
                "bkv": bkv,
                "bq": bq_,
            }
        )
    res = bass_utils.run_bass_kernel_spmd(nc, in_maps, core_ids=list(range(N)))
    out = np.stack([res.results[i]["out"].reshape(C, 64, 64) for i in range(N)])
    return out.astype(np.float32)


if __name__ == "__main__":
    rng = np.random.default_rng(0)
    x = rng.standard_normal((8, C, 64, 64), dtype=np.float32)
    print("built", build_kernel())



# revision 30
# speedup vs baseline: 1.0952x; 1.0952x over previous
"""NonLocalAttention2D Trainium2 kernel (v2).

Data-parallel over batch N=8: one image per NeuronCore.

Per-core math (x: (C=128, HW=4096) fp32):
  kv   = [Wk|0|Wv].T @ x            (96, 4096)  PE fp16 (k rows 0:16, v rows 32:96)
  pool = maxpool2x2(kv)+bias        (96, 1024)  DVE -> kvb fp16
  A_c  = Wq @ k_c                   (128, 128)  PE fp16 -> ab fp16
  bqk  = k.T @ bq, ebqk = exp(bqk)  (128, 8)    PE + ACT
  vaugT= [vT*ebqk | ebqk]           (128, 8*65) PE transpose + DVE -> bf16
  s_cb = A_c.T @ x_b                (128k,512q) PE fp16 -> psum
  attn = exp(s)                     ACT -> bf16 sbuf
  av   = vaugT.T @ attn  (accum 8c) (65, 512)   PE bf16; row 64 = denom
  r    = recip_approx_fast(denom)   (1, 512)    DVE, cast bf16
  R65  = ones65.T @ r               (65, 512)   PE ones-broadcast -> psum
  aoTn = av * R65                   (65, 512)   DVE -> fp16 (row 64 == 1)
  fin  = [g*Wo; g*bo].T @ aoTn      (128, 512)  PE fp16
  out  = fin + x_b                  (128, 512)  DVE -> DMA out
"""

import sys

if "/opt/trn_rl_repo" not in sys.path:
    sys.path.insert(0, "/opt/trn_rl_repo")

import numpy as np

import concourse.bacc as bacc
import concourse.bass as bass
import concourse.tile as tile
from concourse import bass_utils, masks, mybir

F32 = mybir.dt.float32
F16 = mybir.dt.float16
BF16 = mybir.dt.bfloat16
F32R = mybir.dt.float32r

C = 128          # channels
HW = 4096        # 64*64 pixels
L = 1024         # pooled keys (32*32)
D = 16           # attn dim
DV = 64          # value dim
KV = 80          # kv projection out width (v rows 0:64, k rows 64:80)
QB = 512         # q-block size
NB = HW // QB    # 8 q blocks
KC = 128         # keys per chunk
NCH = L // KC    # 8 key chunks
NCORES = 8
WBW = KV + C + C + 1 + DV  # weight blob width: w_kv | wqt | wfin | bq | ident64


def build_kernel(variant="full"):
    nc = bacc.Bacc("TRN2", target_bir_lowering=False, debug=False)

    x_d = nc.dram_tensor("x", (C, HW), F32, kind="ExternalInput").ap()
    wb_d = nc.dram_tensor("wb", (C, WBW), F16, kind="ExternalInput").ap()
    bkv_d = nc.dram_tensor("bkv", (KV, 1), F32, kind="ExternalInput").ap()
    out_d = nc.dram_tensor("out", (C, HW), F32, kind="ExternalOutput").ap()

    from contextlib import ExitStack

    with tile.TileContext(nc) as tc, ExitStack() as ctx:
        singles = ctx.enter_context(tc.tile_pool(name="singles", bufs=1))
        s1_pool = ctx.enter_context(tc.tile_pool(name="s1", bufs=2))
        attn_pool = ctx.enter_context(tc.tile_pool(name="attn", bufs=2))
        r_pool = ctx.enter_context(tc.tile_pool(name="r", bufs=2))
        ao_pool = ctx.enter_context(tc.tile_pool(name="ao", bufs=2))
        out_pool = ctx.enter_context(tc.tile_pool(name="outp", bufs=3))
        dram_pool = ctx.enter_context(tc.tile_pool(name="dram", bufs=2, space="DRAM"))

        ps_sc = ctx.enter_context(tc.tile_pool(name="ps_sc", bufs=2, space="PSUM"))
        ps_av = ctx.enter_context(tc.tile_pool(name="ps_av", bufs=2, space="PSUM"))
        ps_fin = ctx.enter_context(tc.tile_pool(name="ps_fin", bufs=2, space="PSUM"))

        # ---- SBUF singles ----
        wb = singles.tile([C, WBW], F16, tag="wb")
        bkvf = singles.tile([KV, 1], F32, tag="bkvf")
        xf = singles.tile([C, HW], F32, tag="xf")
        xh = singles.tile([C, HW], F16, tag="xh")
        kvf = singles.tile([KV, L], F32, tag="kvf")
        kf = singles.tile([D, L], F32, tag="kf")
        kb = singles.tile([D, L], F16, tag="kb")

        w_kv = wb[:, 0:KV]
        w_qt = wb[0:D, KV : KV + C]
        w_fin = wb[0 : DV + 1, KV + C : KV + 2 * C]
        b_q = wb[0:D, KV + 2 * C : KV + 2 * C + 1]
        ici = KV + 2 * C + 1
        identf = singles.tile([DV, DV], F32, tag="identf")

        # ---- DMAs first so transfers start immediately ----
        nc.sync.dma_start(out=wb, in_=wb_d)
        nc.sync.dma_start(out=bkvf, in_=bkv_d)
        for g in range(4):
            sl = slice(g * 1024, (g + 1) * 1024)
            nc.sync.dma_start(out=xf[:, sl], in_=x_d[:, sl])


        nc.vector.tensor_copy(identf[:, :], wb[0:DV, ici : ici + DV])

        # x -> fp16 casts, one per DMA piece
        for g in range(4):
            sl = slice(g * 1024, (g + 1) * 1024)
            nc.vector.tensor_copy(xh[:, sl], xf[:, sl])

        # ---- prologue: kv proj + pool + bias -> kvb; A_c; bqk; vT ----
        if variant not in ("p1", "p2"):
            vt_t = ps_fin.tile([C, QB], F32, tag="fin")  # 8 x (128,64) vT chunks
        if variant != "p1":
            bqk_t = ps_fin.tile([C, QB], F32, tag="fin")  # cols 0:8 used

        for t in range(4):  # two 512-chunks per psum tile
            proj = ps_sc.tile([KC, 2 * QB], F32, tag="sc")
            for j in range(2):
                c = 2 * t + j
                sl = slice(c * QB, (c + 1) * QB)
                nc.tensor.matmul(
                    proj[:KV, j * QB : (j + 1) * QB],
                    lhsT=w_kv,
                    rhs=xh[:, sl],
                    start=True,
                    stop=True,
                )
            for j in range(2):
                c = 2 * t + j
                csl = slice(c * KC, (c + 1) * KC)
                # maxpool 2x2: w-pairs then h-pairs (chunk covers 8 rows x 64)
                pv = proj[:KV, j * QB : (j + 1) * QB].rearrange(
                    "p (w two) -> p w two", two=2
                )
                s1 = s1_pool.tile([KV, 256], F32, tag="s1")
                nc.vector.tensor_copy(s1[:, :], pv[:, :, 0])
                nc.vector.tensor_max(s1[:, :], s1[:, :], pv[:, :, 1])
                sv = s1.rearrange("p (h two w) -> p h two w", h=4, two=2)
                s2 = s1_pool.tile([KV, KC], F32, tag="s2")
                nc.vector.tensor_max(s2[:, :], sv[:, :, 0, :], sv[:, :, 1, :])
                nc.vector.tensor_scalar_add(kvf[:, csl], s2[:, :], bkvf[:, :])

        if variant == "p1":
            nc.sync.dma_start(out=out_d[:KV, 0:1024], in_=kvf[:, :])
        nc.sync.dma_start(out=kf[:, :], in_=kvf[DV : DV + D, :])
        nc.vector.tensor_copy(kb[:, :], kf[:, :])
        ab = None
        if variant != "p1":
            ab = singles.tile([C, L], F16, tag="ab")
        if variant == "p1":
            a_t = None
        else:
            a_t = ps_sc.tile([KC, 2 * QB], F32, tag="sc")  # all 8 A_c chunks
        for c in range(NCH if variant != "p1" else 0):
            csl = slice(c * KC, (c + 1) * KC)
            # A_c = Wq @ k_c   (k rows 0:16 of kvb)
            nc.tensor.matmul(
                a_t[:, c * KC : (c + 1) * KC],
                lhsT=w_qt,
                rhs=kb[:, csl],
                start=True,
                stop=True,
            )
            # bqk_c = k_c.T @ bq
            nc.tensor.matmul(
                bqk_t[:, c : c + 1], lhsT=kb[:, csl], rhs=b_q,
                start=True, stop=True,
            )
            # vT_c (128, 64) f32 via PE transpose of v rows 0:64
            if variant != "p2":
                nc.tensor.transpose(
                    vt_t[:, c * DV : (c + 1) * DV], kvf[0:DV, csl], identf
                )
        if variant != "p1":
            nc.vector.tensor_copy(ab[:, :], a_t[:, :])

        if variant != "p1":
            ebqk = singles.tile([KC, NCH], F32, tag="ebqk")
            nc.scalar.activation(
                ebqk[:, :], bqk_t[:, 0:NCH], mybir.ActivationFunctionType.Exp
            )
        if variant not in ("p1", "p2", "p3"):
            vaug = singles.tile([KC, NCH * (DV + 1)], BF16, tag="vaug")
        for c in range(NCH if variant not in ("p1", "p2", "p3") else 0):
            base = c * (DV + 1)
            nc.vector.tensor_scalar_mul(
                vaug[:, base : base + DV],
                vt_t[:, c * DV : (c + 1) * DV],
                ebqk[:, c : c + 1],
            )
            nc.vector.tensor_copy(
                vaug[:, base + DV : base + DV + 1], ebqk[:, c : c + 1]
            )

        if variant in ("p2", "p3"):
            nc.sync.dma_start(out=out_d[:, 1024:1536], in_=ab[:, :].bitcast(F32))
        if variant == "prologue":
            nc.sync.dma_start(out=out_d[:KV, 0:1024], in_=kvf[:, :])
            nc.sync.dma_start(out=out_d[:, 1024:1536], in_=ab[:, :].bitcast(F32))
            nc.sync.dma_start(out=out_d[:KC, 2048:2308], in_=vaug[:, :].bitcast(F32))

        # ---- main loop: 4-deep software pipeline ----
        # iter i: PE [sc(i) x8 | av(i-1) x8 | R65MM(i-2) | fin(i-3)]
        #         ACT [exp(i) x4]
        #         DVE [recip(i-1), aoTn-mul(i-2), residual-add(i-3)]
        #         DMA [R65 psum->sbuf (i-2), out (i-3)]
        attn_t, av_t, r_t, R65s_t, fin_t, ao_t = {}, {}, {}, {}, {}, {}

        n_iter = NB + 4 if variant == "full" else (5 if variant == "one" else 0)
        for i in range(n_iter):
            b_sc = i          # scores + exp
            b_av = i - 1      # av accumulation + recip
            b_r = i - 2       # broadcast + aoTn mul
            b_f = i - 3       # fin + residual + store

            if b_sc < (NB if variant == "full" else 1):
                qsl = slice(b_sc * QB, (b_sc + 1) * QB)
                attn = attn_pool.tile([KC, NCH * QB], BF16, tag="attn")
                attn_t[b_sc] = attn
                for t in range(4):
                    sc = ps_sc.tile([KC, 2 * QB], F32, tag="sc")
                    for j in range(2):
                        c = 2 * t + j
                        nc.tensor.matmul(
                            sc[:, j * QB : (j + 1) * QB],
                            lhsT=ab[:, c * KC : (c + 1) * KC],
                            rhs=xh[:, qsl],
                            start=True,
                            stop=True,
                        )
                    # interleave av MMs of previous block between score tiles
                    if t == 1 and 0 <= b_av < NB and b_av in attn_t:
                        _av_mms(nc, ps_av, av_t, vaug, attn_t, b_av, 0, 4)
                    if t == 2 and 0 <= b_av < NB and b_av in attn_t:
                        _av_mms(nc, ps_av, av_t, vaug, attn_t, b_av, 4, 8)
                    nc.scalar.activation(
                        attn[:, t * 2 * QB : (t + 1) * 2 * QB],
                        sc[:, :],
                        mybir.ActivationFunctionType.Exp,
                    )
            elif 0 <= b_av < NB and b_av in attn_t:
                _av_mms(nc, ps_av, av_t, vaug, attn_t, b_av, 0, 8)

            if 0 <= b_av < NB and b_av in attn_t:
                # recip of denominators as soon as av(b_av) stops
                # (custom-DVE recip must read SBUF: stage the psum row first)
                dn = r_pool.tile([1, QB], F32, tag="dn", name=f"dn{b_av}")
                nc.vector.tensor_copy(dn[:, :], av_t[b_av][DV : DV + 1, :])
                r = r_pool.tile([1, QB], F32, tag="r", name=f"r{b_av}")
                nc.vector.reciprocal_approx_fast(r[:, :], dn[:, :])
                r_t[b_av] = r

            if 0 <= b_r < NB and b_r in r_t:
                # broadcast r over 65 partitions via DRAM bounce (partition
                # stride 0 on the read side); hidden by the 4-deep pipeline
                r_dram = dram_pool.tile([1, QB], F32, tag="rd", name=f"rd{b_r}")
                nc.sync.dma_start(out=r_dram[:, :], in_=r_t[b_r][:, :])
                R65s = r_pool.tile([DV + 1, QB], F32, tag="R65s", name=f"R65s{b_r}")
                r_bcast = bass.AP(
                    tensor=r_dram.tensor,
                    offset=r_dram.offset,
                    ap=[[0, DV + 1], [1, QB]],
                )
                nc.sync.dma_start(out=R65s[:, :], in_=r_bcast)
                R65s_t[b_r] = R65s
                ao = ao_pool.tile([DV + 1, QB], F16, tag="ao")
                ao_t[b_r] = ao
                nc.vector.tensor_mul(ao[:, :], av_t[b_r][:, :], R65s[:, :])

            if variant == "one" and b_f == 0:
                nc.sync.dma_start(out=out_d[:, 512:2560], in_=attn_t[0][:, :].bitcast(F32))
                nc.sync.dma_start(out=out_d[:DV + 1, 2560:2816], in_=ao_t[0][:, :].bitcast(F32))
                nc.sync.dma_start(out=out_d[0:1, 2816:3328], in_=r_t[0][:, :])
                nc.sync.dma_start(out=out_d[:DV + 1, 3328:3840], in_=R65s_t[0][:, :])
            if 0 <= b_f < NB and b_f in ao_t:
                qsl = slice(b_f * QB, (b_f + 1) * QB)
                fin = ps_fin.tile([C, QB], F32, tag="fin")
                nc.tensor.matmul(
                    fin[:, :], lhsT=w_fin, rhs=ao_t[b_f][:, :], start=True, stop=True
                )
                o = out_pool.tile([C, QB], F32, tag="o")
                nc.vector.tensor_add(o[:, :], fin[:, :], xf[:, qsl])
                nc.sync.dma_start(out=out_d[:, qsl], in_=o[:, :])

    nc.compile()
    return nc


def _av_mms(nc, ps_av, av_t, vaug, attn_t, b, c0, c1):
    if b not in av_t:
        av_t[b] = ps_av.tile([DV + 1, QB], F32, tag="av", name=f"av{b}")
    av = av_t[b]
    attn = attn_t[b]
    for c in range(c0, c1):
        base = c * (DV + 1)
        nc.tensor.matmul(
            av[:, :],
            lhsT=vaug[:, base : base + DV + 1],
            rhs=attn[:, c * QB : (c + 1) * QB],
            start=(c == 0),
            stop=(c == NCH - 1),
        )


def prep_weights(Wq, bq, Wk, bk, Wv, bv, Wo, bo, gamma):
    g = np.float32(np.asarray(gamma))
    Wq, Wk, Wv, Wo = (np.asarray(a, np.float32) for a in (Wq, Wk, Wv, Wo))
    bq_, bk_, bv_, bo_ = (np.asarray(a, np.float32) for a in (bq, bk, bv, bo))
    wblob = np.zeros((C, WBW), np.float16)
    wblob[:, 0:DV] = Wv.astype(np.float16)           # v cols 0:64
    wblob[:, DV : DV + D] = Wk.astype(np.float16)    # k cols 64:80
    wblob[0:D, KV : KV + C] = Wq.T.astype(np.float16)
    wblob[0:DV, KV + C : KV + 2 * C] = (g * Wo).astype(np.float16)
    wblob[DV, KV + C : KV + 2 * C] = (g * bo_).astype(np.float16)
    wblob[0:D, KV + 2 * C] = bq_.astype(np.float16)
    ici = KV + 2 * C + 1
    wblob[0:DV, ici : ici + DV] = np.eye(DV, dtype=np.float16)
    bkv = np.zeros((KV, 1), np.float32)
    bkv[0:DV, 0] = bv_
    bkv[DV : DV + D, 0] = bk_
    return np.ascontiguousarray(wblob), np.ascontiguousarray(bkv)


_NC_CACHE = {}


def kernel(x, Wq, bq, Wk, bk, Wv, bv, Wo, bo, gamma):
    x = np.asarray(x, dtype=np.float32)
    N = x.shape[0]
    assert x.shape == (N, C, 64, 64) and N == NCORES
    wblob, bkv = prep_weights(Wq, bq, Wk, bk, Wv, bv, Wo, bo, gamma)

    if "nc" not in _NC_CACHE:
        _NC_CACHE["nc"] = build_kernel()
    nc = _NC_CACHE["nc"]

    in_maps = []
    for i in range(N):
        in_maps.append(
            {
                "x": np.ascontiguousarray(x[i].reshape(C, HW)),
                "wb": wblob,
                "bkv": bkv,
            }
        )
    res = bass_utils.run_bass_kernel_spmd(nc, in_maps, core_ids=list(range(N)))
    out = np.stack([res.results[i]["out"].reshape(C, 64, 64) for i in range(N)])
    return out.astype(np.float32)


if __name__ == "__main__":
    print("built", build_kernel())


# revision 31
# speedup vs baseline: 1.3015x; 1.1883x over previous
"""NonLocalAttention2D Trainium2 kernel (v2).

Data-parallel over batch N=8: one image per NeuronCore.

Per-core math (x: (C=128, HW=4096) fp32):
  kv   = [Wk|0|Wv].T @ x            (96, 4096)  PE fp16 (k rows 0:16, v rows 32:96)
  pool = maxpool2x2(kv)+bias        (96, 1024)  DVE -> kvb fp16
  A_c  = Wq @ k_c                   (128, 128)  PE fp16 -> ab fp16
  bqk  = k.T @ bq, ebqk = exp(bqk)  (128, 8)    PE + ACT
  vaugT= [vT*ebqk | ebqk]           (128, 8*65) PE transpose + DVE -> bf16
  s_cb = A_c.T @ x_b                (128k,512q) PE fp16 -> psum
  attn = exp(s)                     ACT -> bf16 sbuf
  av   = vaugT.T @ attn  (accum 8c) (65, 512)   PE bf16; row 64 = denom
  r    = recip_approx_fast(denom)   (1, 512)    DVE, cast bf16
  R65  = ones65.T @ r               (65, 512)   PE ones-broadcast -> psum
  aoTn = av * R65                   (65, 512)   DVE -> fp16 (row 64 == 1)
  fin  = [g*Wo; g*bo].T @ aoTn      (128, 512)  PE fp16
  out  = fin + x_b                  (128, 512)  DVE -> DMA out
"""

import sys

if "/opt/trn_rl_repo" not in sys.path:
    sys.path.insert(0, "/opt/trn_rl_repo")

import numpy as np

import concourse.bacc as bacc
import concourse.bass as bass
import concourse.tile as tile
from concourse import bass_utils, masks, mybir

F32 = mybir.dt.float32
F16 = mybir.dt.float16
BF16 = mybir.dt.bfloat16
F32R = mybir.dt.float32r

C = 128          # channels
HW = 4096        # 64*64 pixels
L = 1024         # pooled keys (32*32)
D = 16           # attn dim
DV = 64          # value dim
KV = 80          # kv projection out width (v rows 0:64, k rows 64:80)
QB = 512         # q-block size
NB = HW // QB    # 8 q blocks
KC = 128         # keys per chunk
NCH = L // KC    # 8 key chunks
NCORES = 8
N_WARM = 18     # PE clock-ramp warmup matmuls
WBW = KV + C + C + 1 + DV  # weight blob width: w_kv | wqt | wfin | bq | ident64


def build_kernel(variant="full"):
    nc = bacc.Bacc("TRN2", target_bir_lowering=False, debug=False)

    x_d = nc.dram_tensor("x", (C, HW), F32, kind="ExternalInput").ap()
    wb_d = nc.dram_tensor("wb", (C, WBW), F16, kind="ExternalInput").ap()
    bkv_d = nc.dram_tensor("bkv", (KV, 1), F32, kind="ExternalInput").ap()
    out_d = nc.dram_tensor("out", (C, HW), F32, kind="ExternalOutput").ap()

    from contextlib import ExitStack

    with tile.TileContext(nc) as tc, ExitStack() as ctx:
        singles = ctx.enter_context(tc.tile_pool(name="singles", bufs=1))
        s1_pool = ctx.enter_context(tc.tile_pool(name="s1", bufs=2))
        attn_pool = ctx.enter_context(tc.tile_pool(name="attn", bufs=2))
        r_pool = ctx.enter_context(tc.tile_pool(name="r", bufs=2))
        ao_pool = ctx.enter_context(tc.tile_pool(name="ao", bufs=2))
        out_pool = ctx.enter_context(tc.tile_pool(name="outp", bufs=3))
        dram_pool = ctx.enter_context(tc.tile_pool(name="dram", bufs=2, space="DRAM"))

        ps_sc = ctx.enter_context(tc.tile_pool(name="ps_sc", bufs=2, space="PSUM"))
        ps_av = ctx.enter_context(tc.tile_pool(name="ps_av", bufs=2, space="PSUM"))
        ps_fin = ctx.enter_context(tc.tile_pool(name="ps_fin", bufs=2, space="PSUM"))

        # ---- SBUF singles ----
        wb = singles.tile([C, WBW], F16, tag="wb")
        bkvf = singles.tile([KV, 1], F32, tag="bkvf")
        xf = singles.tile([C, HW], F32, tag="xf")
        xh = singles.tile([C, HW], F16, tag="xh")
        kvf = singles.tile([KV, L], F32, tag="kvf")
        kb = singles.tile([D, L], F16, tag="kb")

        w_kv = wb[:, 0:KV]
        w_qt = wb[0:D, KV : KV + C]
        w_fin = wb[0 : DV + 1, KV + C : KV + 2 * C]
        b_q = wb[0:D, KV + 2 * C : KV + 2 * C + 1]
        ici = KV + 2 * C + 1
        identf = singles.tile([DV, DV], F32, tag="identf")

        # ---- DMAs first so transfers start immediately ----
        nc.sync.dma_start(out=wb, in_=wb_d)
        nc.sync.dma_start(out=bkvf, in_=bkv_d)
        for g in range(4):
            sl = slice(g * 1024, (g + 1) * 1024)
            nc.sync.dma_start(out=xf[:, sl], in_=x_d[:, sl])


        ones65 = singles.tile([1, DV + 1], BF16, tag="ones")
        nc.vector.memset(ones65, 1.0)
        jnk = singles.tile([C, QB], F16, tag="jnk")
        nc.vector.memset(jnk, 0.0)
        for w in range(N_WARM):
            jp = ps_av.tile([DV + 1, QB], F32, tag="av", name=f"jp{w}")
            nc.tensor.matmul(
                jp[0:DV, :], lhsT=jnk[:, 0:DV], rhs=jnk, start=True, stop=True
            )

        nc.vector.tensor_copy(identf[:, :], wb[0:DV, ici : ici + DV])

        # x -> fp16 casts via gpsimd software-DGE cast DMA (keeps DVE free)
        for g in range(4):
            sl = slice(g * 1024, (g + 1) * 1024)
            nc.gpsimd.dma_start(out=xh[:, sl], in_=xf[:, sl])

        # ---- prologue: kv proj + pool + bias -> kvb; A_c; bqk; vT ----
        if variant not in ("p1", "p2"):
            vt_t = ps_fin.tile([C, QB], F32, tag="fin")  # 8 x (128,64) vT chunks
        if variant != "p1":
            bqk_t = ps_fin.tile([C, QB], F32, tag="fin")  # cols 0:8 used

        ab = None
        a_t = None
        if variant != "p1":
            ab = singles.tile([C, L], F16, tag="ab")
            a_t = ps_sc.tile([KC, 2 * QB], F32, tag="sc")  # all 8 A_c chunks

        def per_chunk_tail(c):
            csl = slice(c * KC, (c + 1) * KC)
            # A_c = Wq @ k_c
            nc.tensor.matmul(
                a_t[:, csl], lhsT=w_qt, rhs=kb[:, csl], start=True, stop=True
            )
            nc.vector.tensor_copy(ab[:, csl], a_t[:, csl])
            # bqk_c = k_c.T @ bq
            nc.tensor.matmul(
                bqk_t[:, c : c + 1], lhsT=kb[:, csl], rhs=b_q,
                start=True, stop=True,
            )
            # vT_c (128, 64) f32 via PE transpose of v rows 0:64
            if variant != "p2":
                nc.tensor.transpose(
                    vt_t[:, c * DV : (c + 1) * DV], kvf[0:DV, csl], identf
                )

        for t in range(4):  # two 512-chunks per psum tile
            proj = ps_sc.tile([KC, 2 * QB], F32, tag="sc")
            for j in range(2):
                c = 2 * t + j
                sl = slice(c * QB, (c + 1) * QB)
                nc.tensor.matmul(
                    proj[:KV, j * QB : (j + 1) * QB],
                    lhsT=w_kv,
                    rhs=xh[:, sl],
                    start=True,
                    stop=True,
                )
            for j in range(2):
                c = 2 * t + j
                csl = slice(c * KC, (c + 1) * KC)
                # maxpool 2x2: w-pairs then h-pairs (chunk covers 8 rows x 64)
                pv = proj[:KV, j * QB : (j + 1) * QB].rearrange(
                    "p (w two) -> p w two", two=2
                )
                s1 = s1_pool.tile([KV, 256], F32, tag="s1")
                nc.vector.tensor_copy(s1[:, :], pv[:, :, 0])
                nc.vector.tensor_max(s1[:, :], s1[:, :], pv[:, :, 1])
                sv = s1.rearrange("p (h two w) -> p h two w", h=4, two=2)
                s2 = s1_pool.tile([KV, KC], F32, tag="s2")
                nc.vector.tensor_max(s2[:, :], sv[:, :, 0, :], sv[:, :, 1, :])
                nc.vector.tensor_scalar_add(kvf[:, csl], s2[:, :], bkvf[:, :])
                # k rows 64:80 -> partitions 0:16, cast fp16 (gpsimd cast DMA)
                nc.gpsimd.dma_start(out=kb[:, csl], in_=kvf[DV : DV + D, csl])
                if variant != "p1":
                    per_chunk_tail(c)

        if variant == "p1":
            nc.sync.dma_start(out=out_d[:KV, 0:1024], in_=kvf[:, :])

        if variant != "p1":
            ebqk = singles.tile([KC, NCH], F32, tag="ebqk")
            nc.scalar.activation(
                ebqk[:, :], bqk_t[:, 0:NCH], mybir.ActivationFunctionType.Exp
            )
        if variant not in ("p1", "p2", "p3"):
            vaug = singles.tile([KC, NCH * (DV + 1)], BF16, tag="vaug")
        for c in range(NCH if variant not in ("p1", "p2", "p3") else 0):
            base = c * (DV + 1)
            nc.vector.tensor_scalar_mul(
                vaug[:, base : base + DV],
                vt_t[:, c * DV : (c + 1) * DV],
                ebqk[:, c : c + 1],
            )
            nc.vector.tensor_copy(
                vaug[:, base + DV : base + DV + 1], ebqk[:, c : c + 1]
            )

        if variant in ("p2", "p3"):
            nc.sync.dma_start(out=out_d[:, 1024:1536], in_=ab[:, :].bitcast(F32))
        if variant == "prologue":
            nc.sync.dma_start(out=out_d[:KV, 0:1024], in_=kvf[:, :])
            nc.sync.dma_start(out=out_d[:, 1024:1536], in_=ab[:, :].bitcast(F32))
            nc.sync.dma_start(out=out_d[:KC, 2048:2308], in_=vaug[:, :].bitcast(F32))

        # ---- main loop: 4-deep software pipeline ----
        # iter i: PE [sc(i) x8 | av(i-1) x8 | R65MM(i-2) | fin(i-3)]
        #         ACT [exp(i) x4]
        #         DVE [recip(i-1), aoTn-mul(i-2), residual-add(i-3)]
        #         DMA [R65 psum->sbuf (i-2), out (i-3)]
        attn_t, av_t, r_t, R65s_t, fin_t, ao_t = {}, {}, {}, {}, {}, {}

        n_iter = NB + 4 if variant == "full" else (5 if variant == "one" else 0)
        for i in range(n_iter):
            b_sc = i          # scores + exp
            b_av = i - 1      # av accumulation + recip
            b_r = i - 2       # broadcast + aoTn mul
            b_f = i - 3       # fin + residual + store

            if b_sc < (NB if variant == "full" else 1):
                qsl = slice(b_sc * QB, (b_sc + 1) * QB)
                attn = attn_pool.tile([KC, NCH * QB], BF16, tag="attn")
                attn_t[b_sc] = attn
                for t in range(4):
                    sc = ps_sc.tile([KC, 2 * QB], F32, tag="sc")
                    for j in range(2):
                        c = 2 * t + j
                        nc.tensor.matmul(
                            sc[:, j * QB : (j + 1) * QB],
                            lhsT=ab[:, c * KC : (c + 1) * KC],
                            rhs=xh[:, qsl],
                            start=True,
                            stop=True,
                        )
                    # interleave av MMs of previous block between score tiles
                    if t == 1 and 0 <= b_av < NB and b_av in attn_t:
                        _av_mms(nc, ps_av, av_t, vaug, attn_t, b_av, 0, 4)
                    if t == 2 and 0 <= b_av < NB and b_av in attn_t:
                        _av_mms(nc, ps_av, av_t, vaug, attn_t, b_av, 4, 8)
                    nc.scalar.activation(
                        attn[:, t * 2 * QB : (t + 1) * 2 * QB],
                        sc[:, :],
                        mybir.ActivationFunctionType.Exp,
                    )
            elif 0 <= b_av < NB and b_av in attn_t:
                _av_mms(nc, ps_av, av_t, vaug, attn_t, b_av, 0, 8)

            if 0 <= b_av < NB and b_av in attn_t:
                # recip of denominators as soon as av(b_av) stops
                # (custom-DVE recip must read SBUF: stage the psum row first)
                dn = r_pool.tile([1, QB], F32, tag="dn", name=f"dn{b_av}")
                nc.vector.tensor_copy(dn[:, :], av_t[b_av][DV : DV + 1, :])
                r = r_pool.tile([1, QB], F32, tag="r", name=f"r{b_av}")
                nc.vector.reciprocal_approx_fast(r[:, :], dn[:, :])
                r_t[b_av] = r

            if 0 <= b_r < NB and b_r in r_t:
                R65s = r_pool.tile([DV + 1, QB], F32, tag="R65s", name=f"R65s{b_r}")
                if b_r < NB - 2:
                    # broadcast r over 65 partitions via DRAM bounce (partition
                    # stride 0 on the read); hidden by the 4-deep pipeline
                    r_dram = dram_pool.tile([1, QB], F32, tag="rd", name=f"rd{b_r}")
                    nc.sync.dma_start(out=r_dram[:, :], in_=r_t[b_r][:, :])
                    r_bcast = bass.AP(
                        tensor=r_dram.tensor,
                        offset=r_dram.offset,
                        ap=[[0, DV + 1], [1, QB]],
                    )
                    nc.sync.dma_start(out=R65s[:, :], in_=r_bcast)
                else:
                    # tail blocks: low-latency path, PE ones-matmul broadcast
                    rb = r_pool.tile([1, QB], BF16, tag="rb", name=f"rb{b_r}")
                    nc.vector.tensor_copy(rb[:, :], r_t[b_r][:, :])
                    R65p = ps_fin.tile([C, QB], F32, tag="fin")
                    nc.tensor.matmul(
                        R65p[0 : DV + 1, :], lhsT=ones65, rhs=rb,
                        start=True, stop=True,
                    )
                    nc.vector.tensor_copy(R65s[:, :], R65p[0 : DV + 1, :])
                R65s_t[b_r] = R65s
                ao = ao_pool.tile([DV + 1, QB], F16, tag="ao")
                ao_t[b_r] = ao
                nc.vector.tensor_mul(ao[:, :], av_t[b_r][:, :], R65s[:, :])

            if variant == "one" and b_f == 0:
                nc.sync.dma_start(out=out_d[:, 512:2560], in_=attn_t[0][:, :].bitcast(F32))
                nc.sync.dma_start(out=out_d[:DV + 1, 2560:2816], in_=ao_t[0][:, :].bitcast(F32))
                nc.sync.dma_start(out=out_d[0:1, 2816:3328], in_=r_t[0][:, :])
                nc.sync.dma_start(out=out_d[:DV + 1, 3328:3840], in_=R65s_t[0][:, :])
            if 0 <= b_f < NB and b_f in ao_t:
                qsl = slice(b_f * QB, (b_f + 1) * QB)
                fin = ps_fin.tile([C, QB], F32, tag="fin")
                nc.tensor.matmul(
                    fin[:, :], lhsT=w_fin, rhs=ao_t[b_f][:, :], start=True, stop=True
                )
                o = out_pool.tile([C, QB], F32, tag="o")
                nc.vector.tensor_add(o[:, :], fin[:, :], xf[:, qsl])
                nc.sync.dma_start(out=out_d[:, qsl], in_=o[:, :])

    nc.compile()
    return nc


def _av_mms(nc, ps_av, av_t, vaug, attn_t, b, c0, c1):
    if b not in av_t:
        av_t[b] = ps_av.tile([DV + 1, QB], F32, tag="av", name=f"av{b}")
    av = av_t[b]
    attn = attn_t[b]
    for c in range(c0, c1):
        base = c * (DV + 1)
        nc.tensor.matmul(
            av[:, :],
            lhsT=vaug[:, base : base + DV + 1],
            rhs=attn[:, c * QB : (c + 1) * QB],
            start=(c == 0),
            stop=(c == NCH - 1),
        )


def prep_weights(Wq, bq, Wk, bk, Wv, bv, Wo, bo, gamma):
    g = np.float32(np.asarray(gamma))
    Wq, Wk, Wv, Wo = (np.asarray(a, np.float32) for a in (Wq, Wk, Wv, Wo))
    bq_, bk_, bv_, bo_ = (np.asarray(a, np.float32) for a in (bq, bk, bv, bo))
    wblob = np.zeros((C, WBW), np.float16)
    wblob[:, 0:DV] = Wv.astype(np.float16)           # v cols 0:64
    wblob[:, DV : DV + D] = Wk.astype(np.float16)    # k cols 64:80
    wblob[0:D, KV : KV + C] = Wq.T.astype(np.float16)
    wblob[0:DV, KV + C : KV + 2 * C] = (g * Wo).astype(np.float16)
    wblob[DV, KV + C : KV + 2 * C] = (g * bo_).astype(np.float16)
    wblob[0:D, KV + 2 * C] = bq_.astype(np.float16)
    ici = KV + 2 * C + 1
    wblob[0:DV, ici : ici + DV] = np.eye(DV, dtype=np.float16)
    bkv = np.zeros((KV, 1), np.float32)
    bkv[0:DV, 0] = bv_
    bkv[DV : DV + D, 0] = bk_
    return np.ascontiguousarray(wblob), np.ascontiguousarray(bkv)


_NC_CACHE = {}


def kernel(x, Wq, bq, Wk, bk, Wv, bv, Wo, bo, gamma):
    x = np.asarray(x, dtype=np.float32)
    N = x.shape[0]
    assert x.shape == (N, C, 64, 64) and N == NCORES
    wblob, bkv = prep_weights(Wq, bq, Wk, bk, Wv, bv, Wo, bo, gamma)

    if "nc" not in _NC_CACHE:
        _NC_CACHE["nc"] = build_kernel()
    nc = _NC_CACHE["nc"]

    in_maps = []
    for i in range(N):
        in_maps.append(
            {
                "x": np.ascontiguousarray(x[i].reshape(C, HW)),
                "wb": wblob,
                "bkv": bkv,
            }
        )
    res = bass_utils.run_bass_kernel_spmd(nc, in_maps, core_ids=list(range(N)))
    out = np.stack([res.results[i]["out"].reshape(C, 64, 64) for i in range(N)])
    return out.astype(np.float32)


if __name__ == "__main__":
    print("built", build_kernel())


# revision 32
# speedup vs baseline: 1.3469x; 1.0349x over previous
"""NonLocalAttention2D Trainium2 kernel (v2).

Data-parallel over batch N=8: one image per NeuronCore.

Per-core math (x: (C=128, HW=4096) fp32):
  kv   = [Wk|0|Wv].T @ x            (96, 4096)  PE fp16 (k rows 0:16, v rows 32:96)
  pool = maxpool2x2(kv)+bias        (96, 1024)  DVE -> kvb fp16
  A_c  = Wq @ k_c                   (128, 128)  PE fp16 -> ab fp16
  bqk  = k.T @ bq, ebqk = exp(bqk)  (128, 8)    PE + ACT
  vaugT= [vT*ebqk | ebqk]           (128, 8*65) PE transpose + DVE -> bf16
  s_cb = A_c.T @ x_b                (128k,512q) PE fp16 -> psum
  attn = exp(s)                     ACT -> bf16 sbuf
  av   = vaugT.T @ attn  (accum 8c) (65, 512)   PE bf16; row 64 = denom
  r    = recip_approx_fast(denom)   (1, 512)    DVE, cast bf16
  R65  = ones65.T @ r               (65, 512)   PE ones-broadcast -> psum
  aoTn = av * R65                   (65, 512)   DVE -> fp16 (row 64 == 1)
  fin  = [g*Wo; g*bo].T @ aoTn      (128, 512)  PE fp16
  out  = fin + x_b                  (128, 512)  DVE -> DMA out
"""

import sys

if "/opt/trn_rl_repo" not in sys.path:
    sys.path.insert(0, "/opt/trn_rl_repo")

import numpy as np

import concourse.bacc as bacc
import concourse.bass as bass
import concourse.tile as tile
from concourse import bass_utils, masks, mybir

F32 = mybir.dt.float32
F16 = mybir.dt.float16
BF16 = mybir.dt.bfloat16
F32R = mybir.dt.float32r

C = 128          # channels
HW = 4096        # 64*64 pixels
L = 1024         # pooled keys (32*32)
D = 16           # attn dim
DV = 64          # value dim
KV = 80          # kv projection out width (v rows 0:64, k rows 64:80)
QB = 512         # q-block size
NB = HW // QB    # 8 q blocks
KC = 128         # keys per chunk
NCH = L // KC    # 8 key chunks
NCORES = 8
N_WARM = 12     # PE clock-ramp warmup matmuls
WBW = KV + C + C + 1 + DV  # weight blob width: w_kv | wqt | wfin | bq | ident64


def build_kernel(variant="full"):
    nc = bacc.Bacc("TRN2", target_bir_lowering=False, debug=False)

    x_d = nc.dram_tensor("x", (C, HW), F32, kind="ExternalInput").ap()
    wb_d = nc.dram_tensor("wb", (C, WBW), F16, kind="ExternalInput").ap()
    bkv_d = nc.dram_tensor("bkv", (KV, 1), F32, kind="ExternalInput").ap()
    out_d = nc.dram_tensor("out", (C, HW), F32, kind="ExternalOutput").ap()

    from contextlib import ExitStack

    with tile.TileContext(nc) as tc, ExitStack() as ctx:
        singles = ctx.enter_context(tc.tile_pool(name="singles", bufs=1))
        s1_pool = ctx.enter_context(tc.tile_pool(name="s1", bufs=2))
        attn_pool = ctx.enter_context(tc.tile_pool(name="attn", bufs=2))
        r_pool = ctx.enter_context(tc.tile_pool(name="r", bufs=2))
        ao_pool = ctx.enter_context(tc.tile_pool(name="ao", bufs=2))
        out_pool = ctx.enter_context(tc.tile_pool(name="outp", bufs=3))
        dram_pool = ctx.enter_context(tc.tile_pool(name="dram", bufs=2, space="DRAM"))

        ps_sc = ctx.enter_context(tc.tile_pool(name="ps_sc", bufs=2, space="PSUM"))
        ps_av = ctx.enter_context(tc.tile_pool(name="ps_av", bufs=2, space="PSUM"))
        ps_fin = ctx.enter_context(tc.tile_pool(name="ps_fin", bufs=2, space="PSUM"))

        # ---- SBUF singles ----
        wb = singles.tile([C, WBW], F16, tag="wb")
        bkvf = singles.tile([KV, 1], F32, tag="bkvf")
        xf = singles.tile([C, HW], F32, tag="xf")
        xh = singles.tile([C, HW], F16, tag="xh")
        kvf = singles.tile([KV, L], F32, tag="kvf")
        kb = singles.tile([D, L], F16, tag="kb")

        w_kv = wb[:, 0:KV]
        w_qt = wb[0:D, KV : KV + C]
        w_fin = wb[0 : DV + 1, KV + C : KV + 2 * C]
        b_q = wb[0:D, KV + 2 * C : KV + 2 * C + 1]
        ici = KV + 2 * C + 1
        identf = singles.tile([DV, DV], F32, tag="identf")

        # ---- DMAs first so transfers start immediately ----
        nc.sync.dma_start(out=wb, in_=wb_d)
        nc.sync.dma_start(out=bkvf, in_=bkv_d)
        for g in range(4):
            sl = slice(g * 1024, (g + 1) * 1024)
            nc.sync.dma_start(out=xf[:, sl], in_=x_d[:, sl])


        ones65 = singles.tile([1, DV + 1], BF16, tag="ones")
        nc.vector.memset(ones65, 1.0)
        jnk = singles.tile([C, QB], F16, tag="jnk")
        nc.vector.memset(jnk, 0.0)
        for w in range(N_WARM):
            jp = ps_av.tile([DV + 1, QB], F32, tag="av", name=f"jp{w}")
            nc.tensor.matmul(
                jp[0:DV, 0:256], lhsT=jnk[:, 0:DV], rhs=jnk[:, 0:256],
                start=True, stop=True,
            )

        nc.vector.tensor_copy(identf[:, :], wb[0:DV, ici : ici + DV])

        # x -> fp16 casts on ACT (idle until the main loop) in 512-col slices
        for g in range(8):
            sl = slice(g * QB, (g + 1) * QB)
            nc.scalar.copy(xh[:, sl], xf[:, sl])

        # ---- prologue: kv proj + pool + bias -> kvb; A_c; bqk; vT ----
        if variant not in ("p1", "p2"):
            vt_t = ps_fin.tile([C, QB], F32, tag="fin")  # 8 x (128,64) vT chunks
        if variant != "p1":
            bqk_t = ps_fin.tile([C, QB], F32, tag="fin")  # cols 0:8 used

        ab = None
        a_t = None
        if variant != "p1":
            ab = singles.tile([C, L], F16, tag="ab")
            a_t = ps_sc.tile([KC, 2 * QB], F32, tag="sc")  # all 8 A_c chunks

        def per_chunk_tail(c):
            csl = slice(c * KC, (c + 1) * KC)
            # A_c = Wq @ k_c
            nc.tensor.matmul(
                a_t[:, csl], lhsT=w_qt, rhs=kb[:, csl], start=True, stop=True
            )
            nc.vector.tensor_copy(ab[:, csl], a_t[:, csl])
            # bqk_c = k_c.T @ bq
            nc.tensor.matmul(
                bqk_t[:, c : c + 1], lhsT=kb[:, csl], rhs=b_q,
                start=True, stop=True,
            )
            # vT_c (128, 64) f32 via PE transpose of v rows 0:64
            if variant != "p2":
                nc.tensor.transpose(
                    vt_t[:, c * DV : (c + 1) * DV], kvf[0:DV, csl], identf
                )

        for t in range(4):  # two 512-chunks per psum tile
            proj = ps_sc.tile([KC, 2 * QB], F32, tag="sc")
            for j in range(2):
                c = 2 * t + j
                sl = slice(c * QB, (c + 1) * QB)
                nc.tensor.matmul(
                    proj[:KV, j * QB : (j + 1) * QB],
                    lhsT=w_kv,
                    rhs=xh[:, sl],
                    start=True,
                    stop=True,
                )
            for j in range(2):
                c = 2 * t + j
                csl = slice(c * KC, (c + 1) * KC)
                # maxpool 2x2: w-pairs then h-pairs (chunk covers 8 rows x 64)
                pv = proj[:KV, j * QB : (j + 1) * QB].rearrange(
                    "p (w two) -> p w two", two=2
                )
                s1 = s1_pool.tile([KV, 256], F32, tag="s1")
                nc.vector.tensor_copy(s1[:, :], pv[:, :, 0])
                nc.vector.tensor_max(s1[:, :], s1[:, :], pv[:, :, 1])
                sv = s1.rearrange("p (h two w) -> p h two w", h=4, two=2)
                s2 = s1_pool.tile([KV, KC], F32, tag="s2")
                nc.vector.tensor_max(s2[:, :], sv[:, :, 0, :], sv[:, :, 1, :])
                nc.vector.tensor_scalar_add(kvf[:, csl], s2[:, :], bkvf[:, :])
                # k rows 64:80 -> partitions 0:16, cast fp16 (gpsimd cast DMA)
                nc.gpsimd.dma_start(out=kb[:, csl], in_=kvf[DV : DV + D, csl])
                if variant != "p1":
                    per_chunk_tail(c)

        if variant == "p1":
            nc.sync.dma_start(out=out_d[:KV, 0:1024], in_=kvf[:, :])

        if variant != "p1":
            ebqk = singles.tile([KC, NCH], F32, tag="ebqk")
            nc.scalar.activation(
                ebqk[:, :], bqk_t[:, 0:NCH], mybir.ActivationFunctionType.Exp
            )
        if variant not in ("p1", "p2", "p3"):
            vaug = singles.tile([KC, NCH * (DV + 1)], BF16, tag="vaug")
        for c in range(NCH if variant not in ("p1", "p2", "p3") else 0):
            base = c * (DV + 1)
            nc.vector.tensor_scalar_mul(
                vaug[:, base : base + DV],
                vt_t[:, c * DV : (c + 1) * DV],
                ebqk[:, c : c + 1],
            )
            nc.vector.tensor_copy(
                vaug[:, base + DV : base + DV + 1], ebqk[:, c : c + 1]
            )

        if variant in ("p2", "p3"):
            nc.sync.dma_start(out=out_d[:, 1024:1536], in_=ab[:, :].bitcast(F32))
        if variant == "prologue":
            nc.sync.dma_start(out=out_d[:KV, 0:1024], in_=kvf[:, :])
            nc.sync.dma_start(out=out_d[:, 1024:1536], in_=ab[:, :].bitcast(F32))
            nc.sync.dma_start(out=out_d[:KC, 2048:2308], in_=vaug[:, :].bitcast(F32))

        # ---- main loop: 4-deep software pipeline ----
        # iter i: PE [sc(i) x8 | av(i-1) x8 | R65MM(i-2) | fin(i-3)]
        #         ACT [exp(i) x4]
        #         DVE [recip(i-1), aoTn-mul(i-2), residual-add(i-3)]
        #         DMA [R65 psum->sbuf (i-2), out (i-3)]
        attn_t, av_t, r_t, R65s_t, fin_t, ao_t = {}, {}, {}, {}, {}, {}

        n_iter = NB + 4 if variant == "full" else (5 if variant == "one" else 0)
        for i in range(n_iter):
            b_sc = i          # scores + exp
            b_av = i - 1      # av accumulation + recip
            b_r = i - 2       # broadcast + aoTn mul
            b_f = i - 3       # fin + residual + store

            if b_sc < (NB if variant == "full" else 1):
                qsl = slice(b_sc * QB, (b_sc + 1) * QB)
                attn = attn_pool.tile([KC, NCH * QB], BF16, tag="attn")
                attn_t[b_sc] = attn
                for t in range(4):
                    sc = ps_sc.tile([KC, 2 * QB], F32, tag="sc")
                    for j in range(2):
                        c = 2 * t + j
                        nc.tensor.matmul(
                            sc[:, j * QB : (j + 1) * QB],
                            lhsT=ab[:, c * KC : (c + 1) * KC],
                            rhs=xh[:, qsl],
                            start=True,
                            stop=True,
                        )
                    # interleave av MMs of previous block between score tiles
                    if t == 1 and 0 <= b_av < NB and b_av in attn_t:
                        _av_mms(nc, ps_av, av_t, vaug, attn_t, b_av, 0, 4)
                    if t == 2 and 0 <= b_av < NB and b_av in attn_t:
                        _av_mms(nc, ps_av, av_t, vaug, attn_t, b_av, 4, 8)
                    nc.scalar.activation(
                        attn[:, t * 2 * QB : (t + 1) * 2 * QB],
                        sc[:, :],
                        mybir.ActivationFunctionType.Exp,
                    )
            elif 0 <= b_av < NB and b_av in attn_t:
                _av_mms(nc, ps_av, av_t, vaug, attn_t, b_av, 0, 8)

            if 0 <= b_av < NB and b_av in attn_t:
                # recip of denominators as soon as av(b_av) stops
                # (custom-DVE recip must read SBUF: stage the psum row first)
                dn = r_pool.tile([1, QB], F32, tag="dn", name=f"dn{b_av}")
                nc.vector.tensor_copy(dn[:, :], av_t[b_av][DV : DV + 1, :])
                r = r_pool.tile([1, QB], F32, tag="r", name=f"r{b_av}")
                nc.vector.reciprocal_approx_fast(r[:, :], dn[:, :])
                r_t[b_av] = r

            if 0 <= b_r < NB and b_r in r_t:
                R65s = r_pool.tile([DV + 1, QB], F32, tag="R65s", name=f"R65s{b_r}")
                if b_r < NB - 2:
                    # broadcast r over 65 partitions via DRAM bounce (partition
                    # stride 0 on the read); hidden by the 4-deep pipeline
                    r_dram = dram_pool.tile([1, QB], F32, tag="rd", name=f"rd{b_r}")
                    nc.sync.dma_start(out=r_dram[:, :], in_=r_t[b_r][:, :])
                    r_bcast = bass.AP(
                        tensor=r_dram.tensor,
                        offset=r_dram.offset,
                        ap=[[0, DV + 1], [1, QB]],
                    )
                    nc.sync.dma_start(out=R65s[:, :], in_=r_bcast)
                else:
                    # tail blocks: low-latency path, PE ones-matmul broadcast
                    rb = r_pool.tile([1, QB], BF16, tag="rb", name=f"rb{b_r}")
                    nc.vector.tensor_copy(rb[:, :], r_t[b_r][:, :])
                    R65p = ps_fin.tile([C, QB], F32, tag="fin")
                    nc.tensor.matmul(
                        R65p[0 : DV + 1, :], lhsT=ones65, rhs=rb,
                        start=True, stop=True,
                    )
                    nc.vector.tensor_copy(R65s[:, :], R65p[0 : DV + 1, :])
                R65s_t[b_r] = R65s
                ao = ao_pool.tile([DV + 1, QB], F16, tag="ao")
                ao_t[b_r] = ao
                nc.vector.tensor_mul(ao[:, :], av_t[b_r][:, :], R65s[:, :])

            if variant == "one" and b_f == 0:
                nc.sync.dma_start(out=out_d[:, 512:2560], in_=attn_t[0][:, :].bitcast(F32))
                nc.sync.dma_start(out=out_d[:DV + 1, 2560:2816], in_=ao_t[0][:, :].bitcast(F32))
                nc.sync.dma_start(out=out_d[0:1, 2816:3328], in_=r_t[0][:, :])
                nc.sync.dma_start(out=out_d[:DV + 1, 3328:3840], in_=R65s_t[0][:, :])
            if 0 <= b_f < NB and b_f in ao_t:
                qsl = slice(b_f * QB, (b_f + 1) * QB)
                fin = ps_fin.tile([C, QB], F32, tag="fin")
                nc.tensor.matmul(
                    fin[:, :], lhsT=w_fin, rhs=ao_t[b_f][:, :], start=True, stop=True
                )
                o = out_pool.tile([C, QB], F32, tag="o")
                nc.vector.tensor_add(o[:, :], fin[:, :], xf[:, qsl])
                nc.sync.dma_start(out=out_d[:, qsl], in_=o[:, :])

    nc.compile()
    return nc


def _av_mms(nc, ps_av, av_t, vaug, attn_t, b, c0, c1):
    if b not in av_t:
        av_t[b] = ps_av.tile([DV + 1, QB], F32, tag="av", name=f"av{b}")
    av = av_t[b]
    attn = attn_t[b]
    for c in range(c0, c1):
        base = c * (DV + 1)
        nc.tensor.matmul(
            av[:, :],
            lhsT=vaug[:, base : base + DV + 1],
            rhs=attn[:, c * QB : (c + 1) * QB],
            start=(c == 0),
            stop=(c == NCH - 1),
        )


def prep_weights(Wq, bq, Wk, bk, Wv, bv, Wo, bo, gamma):
    g = np.float32(np.asarray(gamma))
    Wq, Wk, Wv, Wo = (np.asarray(a, np.float32) for a in (Wq, Wk, Wv, Wo))
    bq_, bk_, bv_, bo_ = (np.asarray(a, np.float32) for a in (bq, bk, bv, bo))
    wblob = np.zeros((C, WBW), np.float16)
    wblob[:, 0:DV] = Wv.astype(np.float16)           # v cols 0:64
    wblob[:, DV : DV + D] = Wk.astype(np.float16)    # k cols 64:80
    wblob[0:D, KV : KV + C] = Wq.T.astype(np.float16)
    wblob[0:DV, KV + C : KV + 2 * C] = (g * Wo).astype(np.float16)
    wblob[DV, KV + C : KV + 2 * C] = (g * bo_).astype(np.float16)
    wblob[0:D, KV + 2 * C] = bq_.astype(np.float16)
    ici = KV + 2 * C + 1
    wblob[0:DV, ici : ici + DV] = np.eye(DV, dtype=np.float16)
    bkv = np.zeros((KV, 1), np.float32)
    bkv[0:DV, 0] = bv_
    bkv[DV : DV + D, 0] = bk_
    return np.ascontiguousarray(wblob), np.ascontiguousarray(bkv)


_NC_CACHE = {}


def kernel(x, Wq, bq, Wk, bk, Wv, bv, Wo, bo, gamma):
    x = np.asarray(x, dtype=np.float32)
    N = x.shape[0]
    assert x.shape == (N, C, 64, 64) and N == NCORES
    wblob, bkv = prep_weights(Wq, bq, Wk, bk, Wv, bv, Wo, bo, gamma)

    if "nc" not in _NC_CACHE:
        _NC_CACHE["nc"] = build_kernel()
    nc = _NC_CACHE["nc"]

    in_maps = []
    for i in range(N):
        in_maps.append(
            {
                "x": np.ascontiguousarray(x[i].reshape(C, HW)),
                "wb": wblob,
                "bkv": bkv,
            }
        )
    res = bass_utils.run_bass_kernel_spmd(nc, in_maps, core_ids=list(range(N)))
    out = np.stack([res.results[i]["out"].reshape(C, 64, 64) for i in range(N)])
    return out.astype(np.float32)


if __name__ == "__main__":
    print("built", build_kernel())


# revision 33
# speedup vs baseline: 1.3738x; 1.0199x over previous
"""NonLocalAttention2D Trainium2 kernel (v2).

Data-parallel over batch N=8: one image per NeuronCore.

Per-core math (x: (C=128, HW=4096) fp32):
  kv   = [Wk|0|Wv].T @ x            (96, 4096)  PE fp16 (k rows 0:16, v rows 32:96)
  pool = maxpool2x2(kv)+bias        (96, 1024)  DVE -> kvb fp16
  A_c  = Wq @ k_c                   (128, 128)  PE fp16 -> ab fp16
  bqk  = k.T @ bq, ebqk = exp(bqk)  (128, 8)    PE + ACT
  vaugT= [vT*ebqk | ebqk]           (128, 8*65) PE transpose + DVE -> bf16
  s_cb = A_c.T @ x_b                (128k,512q) PE fp16 -> psum
  attn = exp(s)                     ACT -> bf16 sbuf
  av   = vaugT.T @ attn  (accum 8c) (65, 512)   PE bf16; row 64 = denom
  r    = recip_approx_fast(denom)   (1, 512)    DVE, cast bf16
  R65  = ones65.T @ r               (65, 512)   PE ones-broadcast -> psum
  aoTn = av * R65                   (65, 512)   DVE -> fp16 (row 64 == 1)
  fin  = [g*Wo; g*bo].T @ aoTn      (128, 512)  PE fp16
  out  = fin + x_b                  (128, 512)  DVE -> DMA out
"""

import sys

if "/opt/trn_rl_repo" not in sys.path:
    sys.path.insert(0, "/opt/trn_rl_repo")

import numpy as np

import concourse.bacc as bacc
import concourse.bass as bass
import concourse.tile as tile
from concourse import bass_utils, masks, mybir

F32 = mybir.dt.float32
F16 = mybir.dt.float16
BF16 = mybir.dt.bfloat16
F32R = mybir.dt.float32r

C = 128          # channels
HW = 4096        # 64*64 pixels
L = 1024         # pooled keys (32*32)
D = 16           # attn dim
DV = 64          # value dim
KV = 80          # kv projection out width (v rows 0:64, k rows 64:80)
QB = 512         # q-block size
NB = HW // QB    # 8 q blocks
KC = 128         # keys per chunk
NCH = L // KC    # 8 key chunks
NCORES = 8
N_WARM = 12     # PE clock-ramp warmup matmuls
WBW = KV + C + C + 1 + DV  # weight blob width: w_kv | wqt | wfin | bq | ident64


def build_kernel(variant="full"):
    nc = bacc.Bacc("TRN2", target_bir_lowering=False, debug=False)

    x_d = nc.dram_tensor("x", (C, HW), F32, kind="ExternalInput").ap()
    wb_d = nc.dram_tensor("wb", (C, WBW), F16, kind="ExternalInput").ap()
    bkv_d = nc.dram_tensor("bkv", (KV, 1), F32, kind="ExternalInput").ap()
    out_d = nc.dram_tensor("out", (C, HW), F32, kind="ExternalOutput").ap()

    from contextlib import ExitStack

    with tile.TileContext(nc) as tc, ExitStack() as ctx:
        singles = ctx.enter_context(tc.tile_pool(name="singles", bufs=1))
        s1_pool = ctx.enter_context(tc.tile_pool(name="s1", bufs=2))
        attn_pool = ctx.enter_context(tc.tile_pool(name="attn", bufs=2))
        r_pool = ctx.enter_context(tc.tile_pool(name="r", bufs=2))
        ao_pool = ctx.enter_context(tc.tile_pool(name="ao", bufs=2))
        out_pool = ctx.enter_context(tc.tile_pool(name="outp", bufs=3))
        dram_pool = ctx.enter_context(tc.tile_pool(name="dram", bufs=2, space="DRAM"))

        ps_sc = ctx.enter_context(tc.tile_pool(name="ps_sc", bufs=2, space="PSUM"))
        ps_av = ctx.enter_context(tc.tile_pool(name="ps_av", bufs=2, space="PSUM"))
        ps_fin = ctx.enter_context(tc.tile_pool(name="ps_fin", bufs=2, space="PSUM"))

        # ---- SBUF singles ----
        wb = singles.tile([C, WBW], F16, tag="wb")
        bkvf = singles.tile([KV, 1], F32, tag="bkvf")
        xf = singles.tile([C, HW], F32, tag="xf")
        xh = singles.tile([C, HW], F16, tag="xh")
        kvf = singles.tile([KV, L], F32, tag="kvf")
        kb = singles.tile([D, L], F16, tag="kb")

        w_kv = wb[:, 0:KV]
        w_qt = wb[0:D, KV : KV + C]
        w_fin = wb[0 : DV + 1, KV + C : KV + 2 * C]
        b_q = wb[0:D, KV + 2 * C : KV + 2 * C + 1]
        ici = KV + 2 * C + 1
        identf = singles.tile([DV, DV], F32, tag="identf")

        # ---- DMAs first so transfers start immediately ----
        nc.sync.dma_start(out=wb, in_=wb_d)
        nc.sync.dma_start(out=bkvf, in_=bkv_d)
        for g in range(4):
            sl = slice(g * 1024, (g + 1) * 1024)
            nc.sync.dma_start(out=xf[:, sl], in_=x_d[:, sl])


        ones65 = singles.tile([1, DV + 1], BF16, tag="ones")
        nc.vector.memset(ones65, 1.0)
        jnk = singles.tile([C, QB], F16, tag="jnk")
        nc.vector.memset(jnk, 0.0)
        for w in range(N_WARM):
            jp = ps_av.tile([DV + 1, QB], F32, tag="av", name=f"jp{w}")
            nc.tensor.matmul(
                jp[0:DV, 0:256], lhsT=jnk[:, 0:DV], rhs=jnk[:, 0:256],
                start=True, stop=True,
            )

        nc.vector.tensor_copy(identf[:, :], wb[0:DV, ici : ici + DV])

        def xh_cast(g):
            sl = slice(g * QB, (g + 1) * QB)
            nc.scalar.copy(xh[:, sl], xf[:, sl])

        # first two x->fp16 casts up front (rest interleaved with pooling)
        xh_cast(0)
        xh_cast(1)

        # ---- prologue: kv proj + pool + bias -> kvb; A_c; bqk; vT ----
        if variant not in ("p1", "p2"):
            vt_t = ps_fin.tile([C, QB], F32, tag="fin")  # 8 x (128,64) vT chunks
        if variant != "p1":
            bqk_t = ps_fin.tile([C, QB], F32, tag="fin")  # cols 0:8 used

        ab = None
        a_t = None
        if variant != "p1":
            ab = singles.tile([C, L], F16, tag="ab")
            a_t = ps_sc.tile([KC, 2 * QB], F32, tag="sc")  # all 8 A_c chunks

        def per_chunk_tail(c):
            csl = slice(c * KC, (c + 1) * KC)
            # A_c = Wq @ k_c
            nc.tensor.matmul(
                a_t[:, csl], lhsT=w_qt, rhs=kb[:, csl], start=True, stop=True
            )
            nc.vector.tensor_copy(ab[:, csl], a_t[:, csl])
            # bqk_c = k_c.T @ bq
            nc.tensor.matmul(
                bqk_t[:, c : c + 1], lhsT=kb[:, csl], rhs=b_q,
                start=True, stop=True,
            )
            # vT_c (128, 64) f32 via PE transpose of v rows 0:64
            if variant != "p2":
                nc.tensor.transpose(
                    vt_t[:, c * DV : (c + 1) * DV], kvf[0:DV, csl], identf
                )

        proj = None
        for c in range(NCH):
            j = c % 2
            if j == 0:
                proj = ps_sc.tile([KC, 2 * QB], F32, tag="sc", name=f"proj{c}")
            sl = slice(c * QB, (c + 1) * QB)
            nc.tensor.matmul(
                proj[:KV, j * QB : (j + 1) * QB],
                lhsT=w_kv,
                rhs=xh[:, sl],
                start=True,
                stop=True,
            )
            csl = slice(c * KC, (c + 1) * KC)
            # maxpool 2x2: w-pairs then h-pairs (chunk covers 8 rows x 64)
            pv = proj[:KV, j * QB : (j + 1) * QB].rearrange(
                "p (w two) -> p w two", two=2
            )
            s1 = s1_pool.tile([KV, 256], F32, tag="s1")
            nc.scalar.copy(s1[:, :], pv[:, :, 0])  # ACT does the psum read
            if c + 2 < NCH:
                xh_cast(c + 2)
            nc.vector.tensor_max(s1[:, :], s1[:, :], pv[:, :, 1])
            sv = s1.rearrange("p (h two w) -> p h two w", h=4, two=2)
            s2 = s1_pool.tile([KV, KC], F32, tag="s2")
            nc.vector.tensor_max(s2[:, :], sv[:, :, 0, :], sv[:, :, 1, :])
            nc.vector.tensor_scalar_add(kvf[:, csl], s2[:, :], bkvf[:, :])
            # k rows 64:80 -> partitions 0:16, cast fp16 (gpsimd cast DMA)
            nc.gpsimd.dma_start(out=kb[:, csl], in_=kvf[DV : DV + D, csl])
            if variant != "p1":
                per_chunk_tail(c)

        if variant == "p1":
            nc.sync.dma_start(out=out_d[:KV, 0:1024], in_=kvf[:, :])

        if variant != "p1":
            ebqk = singles.tile([KC, NCH], F32, tag="ebqk")
            nc.scalar.activation(
                ebqk[:, :], bqk_t[:, 0:NCH], mybir.ActivationFunctionType.Exp
            )
        if variant not in ("p1", "p2", "p3"):
            vaug = singles.tile([KC, NCH * (DV + 1)], BF16, tag="vaug")
        for c in range(NCH if variant not in ("p1", "p2", "p3") else 0):
            base = c * (DV + 1)
            nc.vector.tensor_scalar_mul(
                vaug[:, base : base + DV],
                vt_t[:, c * DV : (c + 1) * DV],
                ebqk[:, c : c + 1],
            )
            nc.vector.tensor_copy(
                vaug[:, base + DV : base + DV + 1], ebqk[:, c : c + 1]
            )

        if variant in ("p2", "p3"):
            nc.sync.dma_start(out=out_d[:, 1024:1536], in_=ab[:, :].bitcast(F32))
        if variant == "prologue":
            nc.sync.dma_start(out=out_d[:KV, 0:1024], in_=kvf[:, :])
            nc.sync.dma_start(out=out_d[:, 1024:1536], in_=ab[:, :].bitcast(F32))
            nc.sync.dma_start(out=out_d[:KC, 2048:2308], in_=vaug[:, :].bitcast(F32))

        # ---- main loop: 4-deep software pipeline ----
        # iter i: PE [sc(i) x8 | av(i-1) x8 | R65MM(i-2) | fin(i-3)]
        #         ACT [exp(i) x4]
        #         DVE [recip(i-1), aoTn-mul(i-2), residual-add(i-3)]
        #         DMA [R65 psum->sbuf (i-2), out (i-3)]
        attn_t, av_t, r_t, R65s_t, fin_t, ao_t = {}, {}, {}, {}, {}, {}

        n_iter = NB + 4 if variant == "full" else (5 if variant == "one" else 0)
        for i in range(n_iter):
            b_sc = i          # scores + exp
            b_av = i - 1      # av accumulation + recip
            b_r = i - 2       # broadcast + aoTn mul
            b_f = i - 3       # fin + residual + store

            if b_sc < (NB if variant == "full" else 1):
                qsl = slice(b_sc * QB, (b_sc + 1) * QB)
                attn = attn_pool.tile([KC, NCH * QB], BF16, tag="attn")
                attn_t[b_sc] = attn
                for t in range(4):
                    sc = ps_sc.tile([KC, 2 * QB], F32, tag="sc")
                    for j in range(2):
                        c = 2 * t + j
                        nc.tensor.matmul(
                            sc[:, j * QB : (j + 1) * QB],
                            lhsT=ab[:, c * KC : (c + 1) * KC],
                            rhs=xh[:, qsl],
                            start=True,
                            stop=True,
                        )
                    # interleave av MMs of previous block between score tiles
                    if t == 1 and 0 <= b_av < NB and b_av in attn_t:
                        _av_mms(nc, ps_av, av_t, vaug, attn_t, b_av, 0, 4)
                    if t == 2 and 0 <= b_av < NB and b_av in attn_t:
                        _av_mms(nc, ps_av, av_t, vaug, attn_t, b_av, 4, 8)
                    nc.scalar.activation(
                        attn[:, t * 2 * QB : (t + 1) * 2 * QB],
                        sc[:, :],
                        mybir.ActivationFunctionType.Exp,
                    )
            elif 0 <= b_av < NB and b_av in attn_t:
                _av_mms(nc, ps_av, av_t, vaug, attn_t, b_av, 0, 8)

            if 0 <= b_av < NB and b_av in attn_t:
                # recip of denominators as soon as av(b_av) stops
                # (custom-DVE recip must read SBUF: stage the psum row first)
                dn = r_pool.tile([1, QB], F32, tag="dn", name=f"dn{b_av}")
                nc.vector.tensor_copy(dn[:, :], av_t[b_av][DV : DV + 1, :])
                r = r_pool.tile([1, QB], F32, tag="r", name=f"r{b_av}")
                nc.vector.reciprocal_approx_fast(r[:, :], dn[:, :])
                r_t[b_av] = r

            if 0 <= b_r < NB and b_r in r_t:
                R65s = r_pool.tile([DV + 1, QB], F32, tag="R65s", name=f"R65s{b_r}")
                if b_r < NB - 2:
                    # broadcast r over 65 partitions via DRAM bounce (partition
                    # stride 0 on the read); hidden by the 4-deep pipeline
                    r_dram = dram_pool.tile([1, QB], F32, tag="rd", name=f"rd{b_r}")
                    nc.sync.dma_start(out=r_dram[:, :], in_=r_t[b_r][:, :])
                    r_bcast = bass.AP(
                        tensor=r_dram.tensor,
                        offset=r_dram.offset,
                        ap=[[0, DV + 1], [1, QB]],
                    )
                    nc.sync.dma_start(out=R65s[:, :], in_=r_bcast)
                else:
                    # tail blocks: low-latency path, PE ones-matmul broadcast
                    rb = r_pool.tile([1, QB], BF16, tag="rb", name=f"rb{b_r}")
                    nc.vector.tensor_copy(rb[:, :], r_t[b_r][:, :])
                    R65p = ps_fin.tile([C, QB], F32, tag="fin")
                    nc.tensor.matmul(
                        R65p[0 : DV + 1, :], lhsT=ones65, rhs=rb,
                        start=True, stop=True,
                    )
                    nc.vector.tensor_copy(R65s[:, :], R65p[0 : DV + 1, :])
                R65s_t[b_r] = R65s
                ao = ao_pool.tile([DV + 1, QB], F16, tag="ao")
                ao_t[b_r] = ao
                nc.vector.tensor_mul(ao[:, :], av_t[b_r][:, :], R65s[:, :])

            if variant == "one" and b_f == 0:
                nc.sync.dma_start(out=out_d[:, 512:2560], in_=attn_t[0][:, :].bitcast(F32))
                nc.sync.dma_start(out=out_d[:DV + 1, 2560:2816], in_=ao_t[0][:, :].bitcast(F32))
                nc.sync.dma_start(out=out_d[0:1, 2816:3328], in_=r_t[0][:, :])
                nc.sync.dma_start(out=out_d[:DV + 1, 3328:3840], in_=R65s_t[0][:, :])
            if 0 <= b_f < NB and b_f in ao_t:
                qsl = slice(b_f * QB, (b_f + 1) * QB)
                fin = ps_fin.tile([C, QB], F32, tag="fin")
                nc.tensor.matmul(
                    fin[:, :], lhsT=w_fin, rhs=ao_t[b_f][:, :], start=True, stop=True
                )
                o = out_pool.tile([C, QB], F32, tag="o")
                nc.vector.tensor_add(o[:, :], fin[:, :], xf[:, qsl])
                nc.sync.dma_start(out=out_d[:, qsl], in_=o[:, :])

    nc.compile()
    return nc


def _av_mms(nc, ps_av, av_t, vaug, attn_t, b, c0, c1):
    if b not in av_t:
        av_t[b] = ps_av.tile([DV + 1, QB], F32, tag="av", name=f"av{b}")
    av = av_t[b]
    attn = attn_t[b]
    for c in range(c0, c1):
        base = c * (DV + 1)
        nc.tensor.matmul(
            av[:, :],
            lhsT=vaug[:, base : base + DV + 1],
            rhs=attn[:, c * QB : (c + 1) * QB],
            start=(c == 0),
            stop=(c == NCH - 1),
        )


def prep_weights(Wq, bq, Wk, bk, Wv, bv, Wo, bo, gamma):
    g = np.float32(np.asarray(gamma))
    Wq, Wk, Wv, Wo = (np.asarray(a, np.float32) for a in (Wq, Wk, Wv, Wo))
    bq_, bk_, bv_, bo_ = (np.asarray(a, np.float32) for a in (bq, bk, bv, bo))
    wblob = np.zeros((C, WBW), np.float16)
    wblob[:, 0:DV] = Wv.astype(np.float16)           # v cols 0:64
    wblob[:, DV : DV + D] = Wk.astype(np.float16)    # k cols 64:80
    wblob[0:D, KV : KV + C] = Wq.T.astype(np.float16)
    wblob[0:DV, KV + C : KV + 2 * C] = (g * Wo).astype(np.float16)
    wblob[DV, KV + C : KV + 2 * C] = (g * bo_).astype(np.float16)
    wblob[0:D, KV + 2 * C] = bq_.astype(np.float16)
    ici = KV + 2 * C + 1
    wblob[0:DV, ici : ici + DV] = np.eye(DV, dtype=np.float16)
    bkv = np.zeros((KV, 1), np.float32)
    bkv[0:DV, 0] = bv_
    bkv[DV : DV + D, 0] = bk_
    return np.ascontiguousarray(wblob), np.ascontiguousarray(bkv)


_NC_CACHE = {}


def kernel(x, Wq, bq, Wk, bk, Wv, bv, Wo, bo, gamma):
    x = np.asarray(x, dtype=np.float32)
    N = x.shape[0]
    assert x.shape == (N, C, 64, 64) and N == NCORES
    wblob, bkv = prep_weights(Wq, bq, Wk, bk, Wv, bv, Wo, bo, gamma)

    if "nc" not in _NC_CACHE:
        _NC_CACHE["nc"] = build_kernel()
    nc = _NC_CACHE["nc"]

    in_maps = []
    for i in range(N):
        in_maps.append(
            {
                "x": np.ascontiguousarray(x[i].reshape(C, HW)),
                "wb": wblob,
                "bkv": bkv,
            }
        )
    res = bass_utils.run_bass_kernel_spmd(nc, in_maps, core_ids=list(range(N)))
    out = np.stack([res.results[i]["out"].reshape(C, 64, 64) for i in range(N)])
    return out.astype(np.float32)


if __name__ == "__main__":
    print("built", build_kernel())
